# revision 43
# baseline (speedup 1.0000x reference)
"""Trainium2 Bass kernel for the AdSBHNet holographic-potential problem.

Key idea: all three integrands are analytic on y in [0,1] (the apparent
sqrt singularities at the endpoints cancel), so a 16-node Gauss-Legendre
rule reproduces the reference's 1000-point trapezoid to ~2.6e-5 relative
(the reference's own discretization error) -- measured in float64 against
the jax reference. That shrinks the quadrature grid 62x vs the trapz
baseline.

Sharding: data-parallel over zs across 8 NeuronCores (1024 each). Per
core the grid is [128 partitions = 8 zs-groups x 16 y-nodes, 128 free =
zs within group]. Polynomial grids (gn, fz, t1, gdd'', Pt') are built by
fp32 TensorEngine matmuls (full precision via the LOW/HIGH 2-pass) with
block-diagonal per-group stationaries, split into K<=24 sub-matmuls
accumulating in PSUM (the PE quarter-row-group path is ~3x faster than
K>=32). The rank-1 grids gd = 1 - W4(y)*zs^4 and x = y*(zs-1) come from
tensor_scalar ops with per-partition scalar vectors instead of matmuls.
DVE/ACT/GPSIMD run the short sqrt chain; one f32r matmul with an
all-ones per-group selector reduces all three integrals for all 1024 zs
at once (f32r is safe here: the |element|-mass to |V| amplification is
<= 3, so TF32-level element rounding stays ~1.5e-3); the tiny tail does
the Vc+Vd combine and the shift.

Numerics: the Vd y-weight mismatch (w/sqrt(y) vs w*y*W2) is folded into
the Pt'/gdd'' stationary coefficients (Pt' = Pt*ratio, gdd'' =
gdd'/ratio, ratio = 1/(y^1.5 W2)), so one selector weight serves all
three chunks. Cancellation-free forms: t1 rows vanish as y->0; 1-zd^4 =
(1-zs)*y2*(1+zd+zd^2+zd^3) with the exact (1-zs)*y2 factor folded into
weights/scales. Pt = fzd*gnd as a single polynomial has ~45x coefficient
amplification at zd->0.1, which is why the grid matmuls must be true
fp32, not f32r (TF32-ish): f32r grids fail the 2e-2 gate at small zs.
"""

import math
import numpy as np

B_TOTAL = 8192
NCORES = 8
BPC = B_TOTAL // NCORES          # 1024 zs per core
NY = 16                          # Gauss-Legendre nodes
G = 8                            # zs groups per core
JC = BPC // G                    # 128 zs per group (free dim)

# cst88 column layout (connected data first so DMA chunk 1 = cols 0:512
# unblocks the PE immediately): rhsC | lhsTC | rhsD | lhsTD
RC0 = 0            # rhsC [48 rows, 128]
LC0 = 128          # lhsTC [48 rows, 3*128]  (gn | fz | t1)
RD0 = 512          # rhsD [72 rows, 128]
LD0 = 640          # lhsTD [72 rows, 2*128]  (gdd'' | Pt')
CW = 896           # cst88 width

# cst128 column layout (early-needed data in chunk A = cols 0:272)
Z40 = 0            # zs^4 replicated [128, 128]
UR0 = 128          # (zs-1) replicated [128, 128]
W4C = 256          # -W4(y) per-partition column
EC = 257           # y(p) per-partition column
SEL0 = 264         # selector [128, 8]
SCL0 = 272         # scl [128, 384]
CT0 = 656          # shift chunk [8, 128] (partitions 0..7)
C2W = 784          # cst128 width

_COMPILED = {}
SPLIT_MM = False


def _build_host_tables(a, b, logcoef, shift, zs):
    """All derived constants in float64, cast to f32 at the end."""
    a = np.asarray(a, np.float64)
    b = np.asarray(b, np.float64)
    lc = float(np.asarray(logcoef).reshape(-1)[0])
    sh = float(np.asarray(shift).reshape(-1)[0])
    zs = np.asarray(zs, np.float64)

    t, wq = np.polynomial.legendre.leggauss(NY)
    y = 0.5 * (t + 1.0)
    wq = 0.5 * wq                         # nodes/weights on [0,1]

    fa1 = 4.0 / 3.0 * a[0]
    fa2 = 2.0 * a[1]
    fa4 = -(1.0 + fa1 + fa2)

    w1 = 1.0 - y * y
    W2 = w1 * w1
    W4 = W2 * W2
    e = y
    ratio = 1.0 / (y ** 1.5 * W2)         # Vd-weight / LVc-weight
    wL = wq * y * W2                      # the single selector weight
    ones = np.ones(NY)

    # connected kinds, 32-aligned blocks: rows 0:24 = {1, z, z2},
    # rows 32:48 = {z4, fs}
    # kind indices: 0='1', 1='z', 2='z2' in block0; 4='z4', 5='fs' in
    # block1 (rows 32:40, 40:48)
    gn_c = {0: ones, 1: b[0] * w1, 2: b[1] * W2}
    fz_c = {0: ones, 1: fa1 * w1, 2: fa2 * W2, 4: fa4 * W4}
    t1_c = {1: fa1 * (w1 - 1), 2: fa2 * (W2 - 1), 4: fa4 * (W4 - 1),
            5: 1.0 - W4}

    # disconnected kinds: rows 0:24 = {1, u, u2}, rows 32:56 = {u3,u4,u5},
    # rows 64:72 = {u6}
    g1 = fa1 + 2 * fa2 + 4 * fa4
    g2 = fa2 + 6 * fa4
    g3 = 4 * fa4
    g4 = fa4
    d0 = 1.0 + b[0] + b[1]
    d1 = b[0] + 2 * b[1]
    d2 = b[1]
    q = np.convolve([0.0, g1, g2, g3, g4], [d0, d1, d2])   # fzd*gnd, powers 0..6

    pt_c = {1: q[1] * e * ratio, 2: q[2] * e**2 * ratio,
            3: q[3] * e**3 * ratio, 4: q[4] * e**4 * ratio,
            5: q[5] * e**5 * ratio, 6: q[6] * e**6 * ratio}
    gd_c = {0: 4 * ones / ratio, 1: 6 * e / ratio, 2: 4 * e**2 / ratio,
            3: e**3 / ratio}

    # kind index -> (row offset) maps
    crow = {0: 0, 1: 8, 2: 16, 4: 32, 5: 40}           # connected, 8 rows/kind
    drow = {0: 0, 1: 8, 2: 16, 3: 32, 4: 40, 5: 48, 6: 64}  # disconnected

    def blockdiag(coefs, rowmap, nrows):
        out = np.zeros((nrows, G * NY))
        for k, cy in coefs.items():
            r0 = rowmap[k]
            for g in range(G):
                out[r0 + g, g * NY:(g + 1) * NY] = cy
        return out

    lhsTC = np.zeros((48, 3 * JC))
    lhsTC[:, 0:JC] = blockdiag(gn_c, crow, 48)
    lhsTC[:, JC:2 * JC] = blockdiag(fz_c, crow, 48)
    lhsTC[:, 2 * JC:3 * JC] = blockdiag(t1_c, crow, 48)
    lhsTD = np.zeros((72, 2 * JC))
    lhsTD[:, 0:JC] = blockdiag(gd_c, drow, 72)
    lhsTD[:, JC:2 * JC] = blockdiag(pt_c, drow, 72)

    cst128_shared = np.zeros((G * NY, C2W))
    for g in range(G):
        cst128_shared[g * NY:(g + 1) * NY, SEL0 + g] = 1.0
        cst128_shared[g * NY:(g + 1) * NY, W4C] = -W4
        cst128_shared[g * NY:(g + 1) * NY, EC] = e

    # ---- per-core zs-derived tables ----
    cst88_all, cst128_all = [], []
    elc = math.exp(lc)
    for c in range(NCORES):
        z = zs[c * BPC:(c + 1) * BPC]
        z2 = z * z
        z4 = z2 * z2
        fs = 1.0 + fa1 * z + fa2 * z2 + fa4 * z4
        u = z - 1.0
        u2 = u * u
        u3 = u2 * u

        def rows(kinds, rowmap, nrows):
            out = np.zeros((nrows, JC))
            for k, kv in kinds.items():
                r0 = rowmap[k]
                out[r0:r0 + G, :] = kv.reshape(G, JC)
            return out

        cst88 = np.zeros((88, CW))
        cst88[0:48, RC0:RC0 + JC] = rows(
            {0: np.ones(BPC), 1: z, 2: z2, 4: z4, 5: fs}, crow, 48)
        cst88[0:72, RD0:RD0 + JC] = rows(
            {0: np.ones(BPC), 1: u, 2: u2, 3: u3, 4: u2 * u2, 5: u2 * u3,
             6: u3 * u3}, drow, 72)
        cst88[0:48, LC0:LC0 + 3 * JC] = lhsTC
        cst88[0:72, LD0:LD0 + 2 * JC] = lhsTD

        cst128 = cst128_shared.copy()
        cL = (4.0 * z * np.sqrt(fs) / math.pi).reshape(G, JC)
        cVc = (4.0 * math.pi * fs * elc / z).reshape(G, JC)
        cVd = (-2.0 * math.pi * np.sqrt(1.0 - z) * elc).reshape(G, JC)
        for g in range(G):
            p = slice(g * NY, (g + 1) * NY)
            cst128[p, SCL0 + 0 * JC:SCL0 + 1 * JC] = wL[:, None] * cL[g][None, :]
            cst128[p, SCL0 + 1 * JC:SCL0 + 2 * JC] = wL[:, None] * cVc[g][None, :]
            cst128[p, SCL0 + 2 * JC:SCL0 + 3 * JC] = wL[:, None] * cVd[g][None, :]
            cst128[p, Z40:Z40 + JC] = z4.reshape(G, JC)[g][None, :]
            cst128[p, UR0:UR0 + JC] = u.reshape(G, JC)[g][None, :]
        cst128[0:G, CT0:CT0 + JC] = sh

        cst88_all.append(cst88.astype(np.float32))
        cst128_all.append(cst128.astype(np.float32))

    return cst88_all, cst128_all


def _patch_tile_drain():
    """Walrus rejects instructions with >4 sync waits; Tile's kernel-tail
    drain waits on every active processor at once. Split it into one drain
    per processor (SP-engine drains are ~12 ns each)."""
    import re as _re
    import concourse.tile as tile_mod
    import bass_rust
    from bass_rust import ScopedClock

    if getattr(tile_mod.TileContext, "_drain_patched", False):
        return

    def _patched(self, tick_clock, wait_clock):
        gc = tick_clock.global_clock
        ticks = [int(x) for x in _re.findall(r"\d+", repr(gc))]
        for i in [i for i, t in enumerate(ticks) if t > 0]:
            sub = bass_rust.VectorClock()
            sub.require_at_least(i, ticks[i])
            d = self.nc.sync.drain()
            wait_clock.add_sem_waits(d.ins, ScopedClock({None: sub}))
        self.nc.all_engine_barrier()
        popped = self.nc._tile_sem_poison_stack.pop()
        assert popped is self._sem_poison
        self.nc.clear_and_free_semaphores(list(self.sems.allocated().values()))
        self.nc.all_engine_barrier()

    tile_mod.TileContext._drain_and_barrier = _patched
    tile_mod.TileContext._drain_patched = True


def _prune_redundant_waits(nc):
    """Tile emits per-instruction sem waits that are not transitively minimal
    (syncing on engine X does not teach it what X itself had waited on), but
    every TPB instruction has exactly ONE sync-wait slot. Run a vector-clock
    closure over the scheduled program, drop every wait already implied by
    the instruction's processor, and hoist any excess waits onto earlier
    same-processor instructions with a free slot (cycle-checked)."""
    insts = []
    for blk in nc.m.functions[0].blocks:
        insts.extend(blk.instructions)

    nonmono = set()
    for inst in insts:
        si = inst.sync_info
        if si is None:
            continue
        for u in si.on_update or []:
            nm = getattr(u, "ant_name", "") or ""
            if getattr(u, "sync_type", "") == "semaphore" and \
                    getattr(u, "update_mode", "") != "sem-inc" and \
                    "barrier" in nm:
                nonmono.add(u.id)
        for w in si.on_wait or []:
            nm = getattr(w, "ant_name", "") or ""
            if "barrier" in nm:
                nonmono.add(w.id)

    V = {}
    snap = {}
    cnt = {}
    own_sem = {}
    free_slots = {}

    def proc_key(inst):
        si = inst.sync_info
        if si is not None:
            for u in si.on_update or []:
                nm = getattr(u, "ant_name", "") or ""
                if nm.startswith("DMA"):
                    return nm
        return str(inst.engine)

    def dep_state(sem, val):
        snaps = snap.get(sem)
        if not snaps:
            return None
        keys = [k for k in snaps if k >= val]
        if not keys:
            return None
        return snaps[min(keys)]

    def merge_from(state, sem, val):
        state[sem] = max(state.get(sem, 0), val)
        ds = dep_state(sem, val)
        if ds:
            for s2, v2 in ds.items():
                if state.get(s2, 0) < v2:
                    state[s2] = v2

    n_dropped = n_hoisted = n_left = 0
    for inst in insts:
        si = inst.sync_info
        pk = proc_key(inst)
        state = V.setdefault(pk, {})
        my_sem = own_sem.get(pk)
        if si is not None and si.on_wait:
            kept = []
            movable = []
            dropped_here = set()
            prestate = dict(state)
            for w in si.on_wait:
                if getattr(w, "sync_type", "") != "semaphore" or \
                        getattr(w, "wait_mode", "") != "sem-ge-imm" or \
                        w.id in nonmono:
                    kept.append(w)
                    continue
                sem, val = w.id, w.wait_value
                # droppable if implied by the processor's prior state or by
                # the transitive closure of the other KEPT waits on this inst
                others = dict(prestate)
                for w2 in si.on_wait:
                    if w2 is w or getattr(w2, "sync_type", "") != "semaphore" \
                            or getattr(w2, "wait_mode", "") != "sem-ge-imm" \
                            or w2.id in nonmono or id(w2) in dropped_here:
                        continue
                    merge_from(others, w2.id, w2.wait_value)
                if others.get(sem, 0) >= val:
                    n_dropped += 1
                    dropped_here.add(id(w))
                else:
                    movable.append(w)
                merge_from(state, sem, val)
            while len(kept) + len(movable) > 1 and movable:
                w = movable.pop(0)
                placed = False
                for tsi, ttick in reversed(free_slots.get(pk, [])):
                    ds = dep_state(w.id, w.wait_value) or {}
                    if my_sem is not None and ds.get(my_sem, 0) >= ttick:
                        continue
                    if not ds:
                        continue
                    tsi.on_wait = [w]
                    free_slots[pk].remove((tsi, ttick))
                    placed = True
                    n_hoisted += 1
                    break
                if not placed:
                    kept.append(w)
                    n_left += 1
            kept.extend(movable)
            if len(kept) != len(si.on_wait):
                si.on_wait = kept
        if si is not None:
            for u in si.on_update or []:
                if getattr(u, "sync_type", "") != "semaphore":
                    continue
                sem = u.id
                if getattr(u, "update_mode", "") != "sem-inc" or sem in nonmono:
                    continue
                uv = getattr(u, "update_value", 1) or 1
                cnt[sem] = cnt.get(sem, 0) + uv
                if not pk.startswith("DMA"):
                    own_sem.setdefault(pk, sem)
                here = dict(state)
                here[sem] = cnt[sem]
                snap.setdefault(sem, {})[cnt[sem]] = here
                state[sem] = cnt[sem]
        if (si is not None and not si.on_wait and not pk.startswith("DMA")
                and str(getattr(inst, "opcode", "")) not in
                ("Matmult", "EventSemaphore", "Drain",
                 "EventSemaphoreRangeClear", "UnconditionalBranch",
                 "CompareBranch", "SetOrderingMode", "Move", "Notify", "Nop")
                and "barrier" not in (inst.name or "")):
            free_slots.setdefault(pk, []).append(
                (si, cnt.get(own_sem.get(pk, -1), 0)))
    if n_left:
        import logging
        logging.warning("_prune_redundant_waits: %d waits could not be "
                        "hoisted; compile may fail", n_left)
    return n_dropped, n_hoisted, n_left


def _act_raw(nc, mybir, func, out, in_, scale=1.0, bias=0.0):
    eng = nc.scalar
    return eng.add_instruction(mybir.InstActivation(
        name=nc.get_next_instruction_name(), func=func,
        ins=[eng.lower_ap(in_),
             mybir.ImmediateValue(dtype=mybir.dt.float32, value=bias),
             mybir.ImmediateValue(dtype=mybir.dt.float32, value=scale),
             mybir.ImmediateValue(dtype=mybir.dt.float32, value=0.0)],
        outs=[eng.lower_ap(out)]))


def _build_nc(prune=True):
    import concourse.bass as bass
    import concourse.mybir as mybir
    from concourse.tile import TileContext
    from concourse.bass import _add_dep_helper

    f32 = mybir.dt.float32
    f32r = mybir.dt.float32r
    AF = mybir.ActivationFunctionType
    ALU = mybir.AluOpType

    _patch_tile_drain()
    nc = bass.Bass(enable_partition_id=False)
    cst88_d = nc.declare_dram_parameter("cst88", [88, CW], f32, isOutput=False)
    cst128_d = nc.declare_dram_parameter("cst128", [128, C2W], f32,
                                         isOutput=False)
    out_d = nc.declare_dram_parameter("out", [G, 2 * JC], f32, isOutput=True)

    with TileContext(nc) as tc:
        with (
            tc.tile_pool(name="const", bufs=1) as cp,
            tc.tile_pool(name="work", bufs=1) as wp,
            tc.tile_pool(name="ps", bufs=1, space="PSUM") as pp,
        ):
            # ---- constants: four DMAs on parallel queues, earliest-needed
            # data first ----
            cs = cp.tile([88, CW], f32)
            nc.sync.dma_start(out=cs[:, 0:512], in_=cst88_d[:, 0:512])
            ck = cp.tile([128, C2W], f32)
            nc.scalar.dma_start(out=ck[:, 0:SCL0], in_=cst128_d[:, 0:SCL0])
            nc.sync.dma_start(out=cs[:, 512:CW], in_=cst88_d[:, 512:CW])
            nc.scalar.dma_start(out=ck[:, SCL0:C2W], in_=cst128_d[:, SCL0:C2W])

            def rc(r0, r1):
                return cs[r0:r1, RC0:RC0 + JC]

            def rd(r0, r1):
                return cs[r0:r1, RD0:RD0 + JC]

            def lc(i, r0, r1):
                return cs[r0:r1, LC0 + i * JC:LC0 + (i + 1) * JC]

            def ld(i, r0, r1):
                return cs[r0:r1, LD0 + i * JC:LD0 + (i + 1) * JC]

            # ---- polynomial grids via PE (fp32 LOW/HIGH, single MM each:
            # fp32 multi-matmul PSUM accumulation hangs the HW) ----
            T1 = pp.tile([128, JC], f32, tag="T1")    # t1
            A = pp.tile([128, 256], f32, tag="A")     # [gn | fz]
            C = pp.tile([128, 256], f32, tag="C")     # [gdd'' | Pt']
            Fp = pp.tile([G, 3 * JC], f32, tag="F")   # reduce output

            MM = dict(skip_group_check=True)
            # t1 first: it gates the whole connected chain
            nc.tensor.matmul(T1[:], lc(2, 0, 48), rc(0, 48),
                             start=True, stop=True, **MM)
            nc.tensor.matmul(A[:, 0:128], lc(0, 0, 24), rc(0, 24),
                             start=True, stop=True, **MM)
            nc.tensor.matmul(A[:, 128:256], lc(1, 0, 40), rc(0, 40),
                             start=True, stop=True, **MM)
            nc.tensor.matmul(C[:, 0:128], ld(0, 0, 40), rd(0, 40),
                             start=True, stop=True, **MM)
            nc.tensor.matmul(C[:, 128:256], ld(1, 0, 72), rd(0, 72),
                             start=True, stop=True, **MM)

            # ---- rank-1 grids from tensor_scalar (per-partition scalars) ----
            GD = wp.tile([128, JC], f32, tag="GD")      # 1 - W4*zs^4
            d1 = nc.vector.tensor_scalar(out=GD[:], in0=ck[:, Z40:Z40 + JC],
                                         scalar1=ck[:, W4C:W4C + 1],
                                         scalar2=1.0, op0=ALU.mult,
                                         op1=ALU.add)
            XD = wp.tile([128, JC], f32, tag="XD")      # x = y*(zs-1)
            d2 = nc.vector.tensor_scalar(out=XD[:], in0=ck[:, UR0:UR0 + JC],
                                         scalar1=ck[:, EC:EC + 1],
                                         scalar2=None, op0=ALU.mult)
            # the f32r reduce needs an f32r-declared stationary
            selr = wp.tile([128, G], f32r, tag="selr")
            d3 = nc.vector.tensor_copy(out=selr[:], in_=ck[:, SEL0:SEL0 + G])

            # ---- elementwise chains.  Emission order = intended per-engine
            # schedule; the a*/d*/p* dep chains pin walrus's static order so
            # a PSUM-gated op cannot stall the critical Rsqrt behind it. ----
            T1S = wp.tile([128, JC], f32, tag="T1S")
            a1 = nc.scalar.copy(out=T1S[:], in_=T1[:])
            Z2S = wp.tile([128, JC], f32, tag="Z2S")
            a2 = _act_raw(nc, mybir, AF.Square, Z2S[:], XD[:],
                          scale=1.0, bias=1.0)
            Z4S = wp.tile([128, JC], f32, tag="Z4S")
            a3 = _act_raw(nc, mybir, AF.Square, Z4S[:], Z2S[:])

            GT = wp.tile([128, JC], f32, tag="GT")      # gd*t1
            d4 = nc.vector.tensor_mul(GT[:], T1[:], GD[:])
            MW = wp.tile([128, 256], f32, tag="MW")     # [gn*gd*t1 | t1*fz]
            d5 = nc.vector.tensor_mul(MW[:, 0:128], A[:, 0:128], GT[:])
            tva = cp.tile([1, 1], f32)
            d6 = nc.vector.tensor_copy(out=tva[:], in_=T1S[0:1, 0:1])
            d7 = nc.vector.tensor_mul(MW[:, 128:256], A[:, 128:256], T1S[:])
            RQW = wp.tile([128, 256], f32, tag="RQW")
            a4 = _act_raw(nc, mybir, AF.Rsqrt, RQW[:], MW[:])
            RRST = wp.tile([128, 3 * JC], f32, tag="RRST")
            d8 = nc.vector.tensor_mul(RRST[:, 0:128], A[:, 0:128],
                                      RQW[:, 0:128])
            ST = wp.tile([128, JC], f32, tag="ST")
            p1 = nc.gpsimd.tensor_mul(ST[:], T1S[:], RQW[:, 128:256])
            USQ = wp.tile([128, JC], f32, tag="USQ")
            a5 = _act_raw(nc, mybir, AF.Square, USQ[:], ST[:],
                          scale=1.0, bias=1.0)
            RDEN = wp.tile([128, JC], f32, tag="RDEN")
            a6 = _act_raw(nc, mybir, AF.Rsqrt, RDEN[:], USQ[:])
            d9 = nc.vector.tensor_mul(RRST[:, 128:256], RRST[:, 0:128],
                                      RDEN[:])

            PTS = wp.tile([128, JC], f32, tag="PTS")
            a7 = nc.scalar.copy(out=PTS[:], in_=C[:, 128:256])
            tvz = cp.tile([1, 1], f32)
            d10 = nc.vector.tensor_copy(out=tvz[:], in_=Z4S[0:1, 0:1])
            G1 = wp.tile([128, JC], f32, tag="G1")
            d11 = nc.vector.tensor_mul(G1[:], C[:, 0:128], Z4S[:])
            tvc = cp.tile([1, 1], f32)
            p2 = nc.gpsimd.tensor_copy(out=tvc[:], in_=G1[0:1, 0:1])
            PG = wp.tile([128, JC], f32, tag="PG")
            p3 = nc.gpsimd.tensor_mul(PG[:], PTS[:], G1[:])
            R2 = wp.tile([128, JC], f32, tag="R2")
            a8 = _act_raw(nc, mybir, AF.Rsqrt, R2[:], PG[:])
            p4 = nc.gpsimd.tensor_mul(RRST[:, 256:384], PTS[:], R2[:])

            # ---- fold weights+scales, then ONE f32r reduce matmul ----
            tvb = cp.tile([1, 1], f32)
            d12 = nc.vector.tensor_copy(out=tvb[:], in_=ck[0:1, SCL0:SCL0 + 1])
            RRS = wp.tile([128, 3 * JC], f32r, tag="RRS")
            d13 = nc.vector.tensor_mul(RRS[:], RRST[:],
                                       ck[:, SCL0:SCL0 + 3 * JC])
            nc.tensor.matmul(Fp[:], selr[:], RRS[:],
                             start=True, stop=True, **MM)

            # ---- tail: Vc+Vd combine, shift, one out DMA ----
            FS = wp.tile([G, 4 * JC], f32, tag="FS")
            a9 = nc.scalar.copy(out=FS[:, 0:3 * JC], in_=Fp[:])
            d14 = nc.vector.tensor_add(FS[:, 3 * JC:4 * JC], FS[:, JC:2 * JC],
                                       FS[:, 2 * JC:3 * JC])
            d15 = nc.vector.tensor_add(FS[:, JC:2 * JC], FS[:, 3 * JC:4 * JC],
                                       ck[0:G, CT0:CT0 + JC])
            nc.sync.dma_start(out=out_d[:], in_=FS[:, 0:2 * JC])

            for chain in ([a1, a2, a3, a4, a5, a6, a7, a8, a9],
                          [d1, d2, d3, d4, d5, d6, d7, d8, d9, d10, d11,
                           d12, d13, d14, d15],
                          [p1, p2, p3, p4]):
                for prev, nxt in zip(chain, chain[1:]):
                    _add_dep_helper(nxt.ins, prev.ins, sync=False,
                                    reason="pin static engine order")

    if prune:
        _prune_redundant_waits(nc)
    return nc


def _get_nc():
    if "nc" not in _COMPILED:
        _COMPILED["nc"] = _build_nc()
    return _COMPILED["nc"]


def kernel(a, b, logcoef, shift, zs, _trace=False):
    from concourse.bass_utils import run_bass_kernel_spmd

    a = np.asarray(a)
    b = np.asarray(b)
    zs = np.asarray(zs)
    assert zs.shape == (B_TOTAL,)

    cst88_all, cst128_all = _build_host_tables(a, b, logcoef, shift, zs)

    in_maps = [
        {"cst88": cst88_all[c], "cst128": cst128_all[c]}
        for c in range(NCORES)
    ]

    nc = _get_nc()
    res = run_bass_kernel_spmd(nc, in_maps, core_ids=list(range(NCORES)),
                               trace=_trace)
    # out [G, 2*JC]: cols 0:128 = L, 128:256 = V, per group g
    outs = []
    for c in range(NCORES):
        o = res.results[c]["out"]
        outs.append(np.stack([o[:, 0:JC].reshape(BPC),
                              o[:, JC:2 * JC].reshape(BPC)]))
    out = np.concatenate(outs, axis=1)
    if _trace:
        kernel.last_exec_time_ns = res.exec_time_ns
        kernel.last_profile = res.profile_json
    return out.astype(np.float32)


# revision 47
# speedup vs baseline: 1.1228x; 1.1228x over previous
"""Trainium2 Bass kernel for the AdSBHNet holographic-potential problem.

Key idea: all three integrands are analytic on y in [0,1] (the apparent
sqrt singularities at the endpoints cancel), so a 16-node Gauss-Legendre
rule reproduces the reference's 1000-point trapezoid to ~2.6e-5 relative
(the reference's own discretization error) -- measured in float64 against
the jax reference. That shrinks the quadrature grid 62x vs the trapz
baseline.

Sharding: data-parallel over zs across 8 NeuronCores (1024 each). Per
core the grid is [128 partitions = 8 zs-groups x 16 y-nodes, 128 free =
zs within group]. Polynomial grids (gn, fz, t1, gdd'', Pt') are built by
fp32 TensorEngine matmuls (full precision via the LOW/HIGH 2-pass) with
block-diagonal per-group stationaries, split into K<=24 sub-matmuls
accumulating in PSUM (the PE quarter-row-group path is ~3x faster than
K>=32). The rank-1 grids gd = 1 - W4(y)*zs^4 and x = y*(zs-1) come from
tensor_scalar ops with per-partition scalar vectors instead of matmuls.
DVE/ACT/GPSIMD run the short sqrt chain; one f32r matmul with an
all-ones per-group selector reduces all three integrals for all 1024 zs
at once (f32r is safe here: the |element|-mass to |V| amplification is
<= 3, so TF32-level element rounding stays ~1.5e-3); the tiny tail does
the Vc+Vd combine and the shift.

Numerics: the Vd y-weight mismatch (w/sqrt(y) vs w*y*W2) is folded into
the Pt'/gdd'' stationary coefficients (Pt' = Pt*ratio, gdd'' =
gdd'/ratio, ratio = 1/(y^1.5 W2)), so one selector weight serves all
three chunks. Cancellation-free forms: t1 rows vanish as y->0; 1-zd^4 =
(1-zs)*y2*(1+zd+zd^2+zd^3) with the exact (1-zs)*y2 factor folded into
weights/scales. Pt = fzd*gnd as a single polynomial has ~45x coefficient
amplification at zd->0.1, which is why the grid matmuls must be true
fp32, not f32r (TF32-ish): f32r grids fail the 2e-2 gate at small zs.
"""

import math
import numpy as np

B_TOTAL = 8192
NCORES = 8
BPC = B_TOTAL // NCORES          # 1024 zs per core
NY = 16                          # Gauss-Legendre nodes
G = 8                            # zs groups per core
JC = BPC // G                    # 128 zs per group (free dim)

# cstA (chunk 1, connected): rhsC | lhsTC -- contiguous DRAM param
RC0 = 0            # rhsC [48 rows, 128]
LC0 = 128          # lhsTC [48 rows, 3*128]  (t1 | fz | gn)
CAW = 512
# cstB (chunk 2, disconnected): rhsD | lhsTD -- contiguous DRAM param
RD0 = 0            # rhsD [72 rows, 128]
LD0 = 128          # lhsTD [72 rows, 2*128]  (gdd'' | Pt')
CBW = 384

# ckA (chunk 3): replicated rows + per-partition columns + selector
Z40 = 0            # zs^4 replicated [128, 128]
UR0 = 128          # (zs-1) replicated [128, 128]
W4C = 256          # -W4(y) per-partition column
EC = 257           # y(p) per-partition column
SEL0 = 264         # selector [128, 8]
CKAW = 272
# ckB (chunk 4): scl + shift
SCL0 = 0           # scl [128, 384]
CT0 = 384          # shift chunk [8, 128] (partitions 0..7)
CKBW = 512

_COMPILED = {}
SPLIT_MM = False


def _build_host_tables(a, b, logcoef, shift, zs):
    """All derived constants in float64, cast to f32 at the end."""
    a = np.asarray(a, np.float64)
    b = np.asarray(b, np.float64)
    lc = float(np.asarray(logcoef).reshape(-1)[0])
    sh = float(np.asarray(shift).reshape(-1)[0])
    zs = np.asarray(zs, np.float64)

    t, wq = np.polynomial.legendre.leggauss(NY)
    y = 0.5 * (t + 1.0)
    wq = 0.5 * wq                         # nodes/weights on [0,1]

    fa1 = 4.0 / 3.0 * a[0]
    fa2 = 2.0 * a[1]
    fa4 = -(1.0 + fa1 + fa2)

    w1 = 1.0 - y * y
    W2 = w1 * w1
    W4 = W2 * W2
    e = y
    ratio = 1.0 / (y ** 1.5 * W2)         # Vd-weight / LVc-weight
    wL = wq * y * W2                      # the single selector weight
    ones = np.ones(NY)

    # connected kinds, 32-aligned blocks: rows 0:24 = {1, z, z2},
    # rows 32:48 = {z4, fs}
    # kind indices: 0='1', 1='z', 2='z2' in block0; 4='z4', 5='fs' in
    # block1 (rows 32:40, 40:48)
    gn_c = {0: ones, 1: b[0] * w1, 2: b[1] * W2}
    fz_c = {0: ones, 1: fa1 * w1, 2: fa2 * W2, 4: fa4 * W4}
    t1_c = {1: fa1 * (w1 - 1), 2: fa2 * (W2 - 1), 4: fa4 * (W4 - 1),
            5: 1.0 - W4}

    # disconnected kinds: rows 0:24 = {1, u, u2}, rows 32:56 = {u3,u4,u5},
    # rows 64:72 = {u6}
    g1 = fa1 + 2 * fa2 + 4 * fa4
    g2 = fa2 + 6 * fa4
    g3 = 4 * fa4
    g4 = fa4
    d0 = 1.0 + b[0] + b[1]
    d1 = b[0] + 2 * b[1]
    d2 = b[1]
    q = np.convolve([0.0, g1, g2, g3, g4], [d0, d1, d2])   # fzd*gnd, powers 0..6

    pt_c = {1: q[1] * e * ratio, 2: q[2] * e**2 * ratio,
            3: q[3] * e**3 * ratio, 4: q[4] * e**4 * ratio,
            5: q[5] * e**5 * ratio, 6: q[6] * e**6 * ratio}
    gd_c = {0: 4 * ones / ratio, 1: 6 * e / ratio, 2: 4 * e**2 / ratio,
            3: e**3 / ratio}

    # kind index -> (row offset) maps
    crow = {0: 0, 1: 8, 2: 16, 4: 32, 5: 40}           # connected, 8 rows/kind
    drow = {0: 0, 1: 8, 2: 16, 3: 32, 4: 40, 5: 48, 6: 64}  # disconnected

    def blockdiag(coefs, rowmap, nrows):
        out = np.zeros((nrows, G * NY))
        for k, cy in coefs.items():
            r0 = rowmap[k]
            for g in range(G):
                out[r0 + g, g * NY:(g + 1) * NY] = cy
        return out

    lhsTC = np.zeros((48, 3 * JC))
    lhsTC[:, 0:JC] = blockdiag(t1_c, crow, 48)
    lhsTC[:, JC:2 * JC] = blockdiag(fz_c, crow, 48)
    lhsTC[:, 2 * JC:3 * JC] = blockdiag(gn_c, crow, 48)
    lhsTD = np.zeros((72, 2 * JC))
    lhsTD[:, 0:JC] = blockdiag(gd_c, drow, 72)
    lhsTD[:, JC:2 * JC] = blockdiag(pt_c, drow, 72)

    ckA_shared = np.zeros((G * NY, CKAW))
    for g in range(G):
        ckA_shared[g * NY:(g + 1) * NY, SEL0 + g] = 1.0
        ckA_shared[g * NY:(g + 1) * NY, W4C] = -W4
        ckA_shared[g * NY:(g + 1) * NY, EC] = e

    # ---- per-core zs-derived tables ----
    cstA_all, cstB_all, ckA_all, ckB_all = [], [], [], []
    elc = math.exp(lc)
    for c in range(NCORES):
        z = zs[c * BPC:(c + 1) * BPC]
        z2 = z * z
        z4 = z2 * z2
        fs = 1.0 + fa1 * z + fa2 * z2 + fa4 * z4
        u = z - 1.0
        u2 = u * u
        u3 = u2 * u

        def rows(kinds, rowmap, nrows):
            out = np.zeros((nrows, JC))
            for k, kv in kinds.items():
                r0 = rowmap[k]
                out[r0:r0 + G, :] = kv.reshape(G, JC)
            return out

        cstA = np.zeros((48, CAW))
        cstA[0:48, RC0:RC0 + JC] = rows(
            {0: np.ones(BPC), 1: z, 2: z2, 4: z4, 5: fs}, crow, 48)
        cstA[0:48, LC0:LC0 + 3 * JC] = lhsTC
        cstB = np.zeros((72, CBW))
        cstB[0:72, RD0:RD0 + JC] = rows(
            {0: np.ones(BPC), 1: u, 2: u2, 3: u3, 4: u2 * u2, 5: u2 * u3,
             6: u3 * u3}, drow, 72)
        cstB[0:72, LD0:LD0 + 2 * JC] = lhsTD

        ckA = ckA_shared.copy()
        ckB = np.zeros((G * NY, CKBW))
        cL = (4.0 * z * np.sqrt(fs) / math.pi).reshape(G, JC)
        cVc = (4.0 * math.pi * fs * elc / z).reshape(G, JC)
        cVd = (-2.0 * math.pi * np.sqrt(1.0 - z) * elc).reshape(G, JC)
        for g in range(G):
            p = slice(g * NY, (g + 1) * NY)
            ckB[p, SCL0 + 0 * JC:SCL0 + 1 * JC] = wL[:, None] * cL[g][None, :]
            ckB[p, SCL0 + 1 * JC:SCL0 + 2 * JC] = wL[:, None] * cVc[g][None, :]
            ckB[p, SCL0 + 2 * JC:SCL0 + 3 * JC] = wL[:, None] * cVd[g][None, :]
            ckA[p, Z40:Z40 + JC] = z4.reshape(G, JC)[g][None, :]
            ckA[p, UR0:UR0 + JC] = u.reshape(G, JC)[g][None, :]
        ckB[0:G, CT0:CT0 + JC] = sh

        cstA_all.append(cstA.astype(np.float32))
        cstB_all.append(cstB.astype(np.float32))
        ckA_all.append(ckA.astype(np.float32))
        ckB_all.append(ckB.astype(np.float32))

    return cstA_all, cstB_all, ckA_all, ckB_all


def _patch_tile_drain():
    """Walrus rejects instructions with >4 sync waits; Tile's kernel-tail
    drain waits on every active processor at once. Split it into one drain
    per processor (SP-engine drains are ~12 ns each)."""
    import re as _re
    import concourse.tile as tile_mod
    import bass_rust
    from bass_rust import ScopedClock

    if getattr(tile_mod.TileContext, "_drain_patched", False):
        return

    def _patched(self, tick_clock, wait_clock):
        gc = tick_clock.global_clock
        ticks = [int(x) for x in _re.findall(r"\d+", repr(gc))]
        for i in [i for i, t in enumerate(ticks) if t > 0]:
            sub = bass_rust.VectorClock()
            sub.require_at_least(i, ticks[i])
            d = self.nc.sync.drain()
            wait_clock.add_sem_waits(d.ins, ScopedClock({None: sub}))
        self.nc.all_engine_barrier()
        popped = self.nc._tile_sem_poison_stack.pop()
        assert popped is self._sem_poison
        self.nc.clear_and_free_semaphores(list(self.sems.allocated().values()))
        self.nc.all_engine_barrier()

    tile_mod.TileContext._drain_and_barrier = _patched
    tile_mod.TileContext._drain_patched = True


def _prune_redundant_waits(nc):
    """Tile emits per-instruction sem waits that are not transitively minimal
    (syncing on engine X does not teach it what X itself had waited on), but
    every TPB instruction has exactly ONE sync-wait slot. Run a vector-clock
    closure over the scheduled program, drop every wait already implied by
    the instruction's processor, and hoist any excess waits onto earlier
    same-processor instructions with a free slot (cycle-checked)."""
    insts = []
    for blk in nc.m.functions[0].blocks:
        insts.extend(blk.instructions)

    nonmono = set()
    for inst in insts:
        si = inst.sync_info
        if si is None:
            continue
        for u in si.on_update or []:
            nm = getattr(u, "ant_name", "") or ""
            if getattr(u, "sync_type", "") == "semaphore" and \
                    getattr(u, "update_mode", "") != "sem-inc" and \
                    "barrier" in nm:
                nonmono.add(u.id)
        for w in si.on_wait or []:
            nm = getattr(w, "ant_name", "") or ""
            if "barrier" in nm:
                nonmono.add(w.id)

    V = {}
    snap = {}
    cnt = {}
    own_sem = {}
    free_slots = {}

    def proc_key(inst):
        si = inst.sync_info
        if si is not None:
            for u in si.on_update or []:
                nm = getattr(u, "ant_name", "") or ""
                if nm.startswith("DMA"):
                    return nm
        return str(inst.engine)

    def dep_state(sem, val):
        snaps = snap.get(sem)
        if not snaps:
            return None
        keys = [k for k in snaps if k >= val]
        if not keys:
            return None
        return snaps[min(keys)]

    def merge_from(state, sem, val):
        state[sem] = max(state.get(sem, 0), val)
        ds = dep_state(sem, val)
        if ds:
            for s2, v2 in ds.items():
                if state.get(s2, 0) < v2:
                    state[s2] = v2

    n_dropped = n_hoisted = n_left = 0
    for inst in insts:
        si = inst.sync_info
        pk = proc_key(inst)
        state = V.setdefault(pk, {})
        my_sem = own_sem.get(pk)
        if si is not None and si.on_wait:
            kept = []
            movable = []
            dropped_here = set()
            prestate = dict(state)
            for w in si.on_wait:
                if getattr(w, "sync_type", "") != "semaphore" or \
                        getattr(w, "wait_mode", "") != "sem-ge-imm" or \
                        w.id in nonmono:
                    kept.append(w)
                    continue
                sem, val = w.id, w.wait_value
                # droppable if implied by the processor's prior state or by
                # the transitive closure of the other KEPT waits on this inst
                others = dict(prestate)
                for w2 in si.on_wait:
                    if w2 is w or getattr(w2, "sync_type", "") != "semaphore" \
                            or getattr(w2, "wait_mode", "") != "sem-ge-imm" \
                            or w2.id in nonmono or id(w2) in dropped_here:
                        continue
                    merge_from(others, w2.id, w2.wait_value)
                if others.get(sem, 0) >= val:
                    n_dropped += 1
                    dropped_here.add(id(w))
                else:
                    movable.append(w)
                merge_from(state, sem, val)
            while len(kept) + len(movable) > 1 and movable:
                w = movable.pop(0)
                placed = False
                for tsi, ttick in reversed(free_slots.get(pk, [])):
                    ds = dep_state(w.id, w.wait_value) or {}
                    if my_sem is not None and ds.get(my_sem, 0) >= ttick:
                        continue
                    if not ds:
                        continue
                    tsi.on_wait = [w]
                    free_slots[pk].remove((tsi, ttick))
                    placed = True
                    n_hoisted += 1
                    break
                if not placed:
                    kept.append(w)
                    n_left += 1
            kept.extend(movable)
            if len(kept) != len(si.on_wait):
                si.on_wait = kept
        if si is not None:
            for u in si.on_update or []:
                if getattr(u, "sync_type", "") != "semaphore":
                    continue
                sem = u.id
                if getattr(u, "update_mode", "") != "sem-inc" or sem in nonmono:
                    continue
                uv = getattr(u, "update_value", 1) or 1
                cnt[sem] = cnt.get(sem, 0) + uv
                if not pk.startswith("DMA"):
                    own_sem.setdefault(pk, sem)
                here = dict(state)
                here[sem] = cnt[sem]
                snap.setdefault(sem, {})[cnt[sem]] = here
                state[sem] = cnt[sem]
        if (si is not None and not si.on_wait and not pk.startswith("DMA")
                and str(getattr(inst, "opcode", "")) not in
                ("Matmult", "EventSemaphore", "Drain",
                 "EventSemaphoreRangeClear", "UnconditionalBranch",
                 "CompareBranch", "SetOrderingMode", "Move", "Notify", "Nop")
                and "barrier" not in (inst.name or "")):
            free_slots.setdefault(pk, []).append(
                (si, cnt.get(own_sem.get(pk, -1), 0)))
    if n_left:
        import logging
        logging.warning("_prune_redundant_waits: %d waits could not be "
                        "hoisted; compile may fail", n_left)
    return n_dropped, n_hoisted, n_left


def _act_raw(nc, mybir, func, out, in_, scale=1.0, bias=0.0):
    eng = nc.scalar
    return eng.add_instruction(mybir.InstActivation(
        name=nc.get_next_instruction_name(), func=func,
        ins=[eng.lower_ap(in_),
             mybir.ImmediateValue(dtype=mybir.dt.float32, value=bias),
             mybir.ImmediateValue(dtype=mybir.dt.float32, value=scale),
             mybir.ImmediateValue(dtype=mybir.dt.float32, value=0.0)],
        outs=[eng.lower_ap(out)]))


def _build_nc(prune=True):
    import concourse.bass as bass
    import concourse.mybir as mybir
    from concourse.tile import TileContext
    from concourse.bass import _add_dep_helper

    f32 = mybir.dt.float32
    f32r = mybir.dt.float32r
    AF = mybir.ActivationFunctionType
    ALU = mybir.AluOpType

    _patch_tile_drain()
    nc = bass.Bass(enable_partition_id=False)
    cstA_d = nc.declare_dram_parameter("cstA", [48, CAW], f32, isOutput=False)
    cstB_d = nc.declare_dram_parameter("cstB", [72, CBW], f32, isOutput=False)
    ckA_d = nc.declare_dram_parameter("ckA", [128, CKAW], f32, isOutput=False)
    ckB_d = nc.declare_dram_parameter("ckB", [128, CKBW], f32, isOutput=False)
    out_d = nc.declare_dram_parameter("out", [G, 2 * JC], f32, isOutput=True)

    with TileContext(nc) as tc:
        with (
            tc.tile_pool(name="const", bufs=1) as cp,
            tc.tile_pool(name="work", bufs=1) as wp,
            tc.tile_pool(name="ps", bufs=1, space="PSUM") as pp,
        ):
            # ---- constants: four contiguous DMAs, two per HWDGE ring ----
            # f32r tiles: the f32r grid matmuls need f32r-declared producers
            ca = cp.tile([48, CAW], f32r)
            nc.sync.dma_start(out=ca[:], in_=cstA_d[:].bitcast(f32r))
            ka = cp.tile([128, CKAW], f32)
            nc.scalar.dma_start(out=ka[:], in_=ckA_d[:])
            cb = cp.tile([72, CBW], f32r)
            nc.sync.dma_start(out=cb[:], in_=cstB_d[:].bitcast(f32r))
            kb = cp.tile([128, CKBW], f32)
            nc.scalar.dma_start(out=kb[:], in_=ckB_d[:])

            def rc(r0, r1):
                return ca[r0:r1, RC0:RC0 + JC]

            def rd(r0, r1):
                return cb[r0:r1, RD0:RD0 + JC]

            def lc(i, r0, r1):
                return ca[r0:r1, LC0 + i * JC:LC0 + (i + 1) * JC]

            def ld(i, r0, r1):
                return cb[r0:r1, LD0 + i * JC:LD0 + (i + 1) * JC]

            # ---- polynomial grids via PE.  t1/fz/gn/gdd are f32r (their
            # coefficient-amplification is <= 5, so TF32 rounding stays
            # ~2e-3); Pt has ~45x amplification and must be true fp32.
            # Single matmul per grid: fp32 multi-matmul PSUM accumulation
            # hangs the HW.  One PSUM tile per grid (whole-tile dep tracking
            # otherwise stalls consumers on unrelated grids). ----
            T1 = pp.tile([128, JC], f32, tag="T1")
            FZ = pp.tile([128, JC], f32, tag="FZ")
            GN = pp.tile([128, JC], f32, tag="GN")
            GDD = pp.tile([128, JC], f32, tag="GDD")
            PT = pp.tile([128, JC], f32, tag="PT")
            Fp = pp.tile([G, 3 * JC], f32, tag="F")   # reduce output

            MM = dict(skip_group_check=True)
            nc.tensor.matmul(T1[:], lc(0, 0, 48), rc(0, 48),
                             start=True, stop=True, **MM)
            nc.tensor.matmul(FZ[:], lc(1, 0, 40), rc(0, 40),
                             start=True, stop=True, **MM)
            nc.tensor.matmul(GN[:], lc(2, 0, 24), rc(0, 24),
                             start=True, stop=True, **MM)
            nc.tensor.matmul(GDD[:], ld(0, 0, 40), rd(0, 40),
                             start=True, stop=True, **MM)
            nc.tensor.matmul(PT[:], ld(1, 0, 72).bitcast(f32),
                             rd(0, 72).bitcast(f32),
                             start=True, stop=True, **MM)

            # ---- rank-1 grids from tensor_scalar (per-partition scalars) ----
            GD = wp.tile([128, JC], f32, tag="GD")      # 1 - W4*zs^4
            d1 = nc.vector.tensor_scalar(out=GD[:], in0=ka[:, Z40:Z40 + JC],
                                         scalar1=ka[:, W4C:W4C + 1],
                                         scalar2=1.0, op0=ALU.mult,
                                         op1=ALU.add)
            XD = wp.tile([128, JC], f32, tag="XD")      # x = y*(zs-1)
            d2 = nc.vector.tensor_scalar(out=XD[:], in0=ka[:, UR0:UR0 + JC],
                                         scalar1=ka[:, EC:EC + 1],
                                         scalar2=None, op0=ALU.mult)
            # the f32r reduce needs an f32r-declared stationary
            selr = wp.tile([128, G], f32r, tag="selr")
            d3 = nc.vector.tensor_copy(out=selr[:], in_=ka[:, SEL0:SEL0 + G])

            # ---- elementwise chains.  Emission order = intended per-engine
            # schedule (consumers of PSUM grids are emitted before any ACT
            # instruction that could become their piggyback dependency); the
            # dep chains pin walrus's static order. ----
            GT = wp.tile([128, JC], f32, tag="GT")      # gd*t1
            d4 = nc.vector.tensor_mul(GT[:], T1[:], GD[:])
            T1S = wp.tile([128, JC], f32, tag="T1S")
            a1 = nc.scalar.copy(out=T1S[:], in_=T1[:])
            MW = wp.tile([128, 256], f32, tag="MW")     # [t1*fz | gn*gd*t1]
            d5 = nc.vector.tensor_mul(MW[:, 0:128], FZ[:], T1S[:])
            d6 = nc.vector.tensor_mul(MW[:, 128:256], GN[:], GT[:])
            RQW = wp.tile([128, 256], f32, tag="RQW")
            a2 = _act_raw(nc, mybir, AF.Rsqrt, RQW[:, 0:128], MW[:, 0:128])
            a3 = _act_raw(nc, mybir, AF.Rsqrt, RQW[:, 128:256],
                          MW[:, 128:256])
            Z2S = wp.tile([128, JC], f32, tag="Z2S")
            a4 = _act_raw(nc, mybir, AF.Square, Z2S[:], XD[:],
                          scale=1.0, bias=1.0)
            Z4S = wp.tile([128, JC], f32, tag="Z4S")
            a5 = _act_raw(nc, mybir, AF.Square, Z4S[:], Z2S[:])

            RRST = wp.tile([128, 3 * JC], f32, tag="RRST")
            d7 = nc.vector.tensor_mul(RRST[:, 0:128], GN[:], RQW[:, 128:256])
            ST = wp.tile([128, JC], f32, tag="ST")
            p1 = nc.gpsimd.tensor_mul(ST[:], T1S[:], RQW[:, 0:128])
            USQ = wp.tile([128, JC], f32, tag="USQ")
            a6 = _act_raw(nc, mybir, AF.Square, USQ[:], ST[:],
                          scale=1.0, bias=1.0)
            RDEN = wp.tile([128, JC], f32, tag="RDEN")
            a7 = _act_raw(nc, mybir, AF.Rsqrt, RDEN[:], USQ[:])
            d8 = nc.vector.tensor_mul(RRST[:, 128:256], RRST[:, 0:128],
                                      RDEN[:])

            G1 = wp.tile([128, JC], f32, tag="G1")
            d9 = nc.vector.tensor_mul(G1[:], GDD[:], Z4S[:])
            PTS = wp.tile([128, JC], f32, tag="PTS")
            a8 = nc.scalar.copy(out=PTS[:], in_=PT[:])
            tvc = cp.tile([1, 1], f32)
            p2 = nc.gpsimd.tensor_copy(out=tvc[:], in_=G1[0:1, 0:1])
            PG = wp.tile([128, JC], f32, tag="PG")
            p3 = nc.gpsimd.tensor_mul(PG[:], PTS[:], G1[:])
            R2 = wp.tile([128, JC], f32, tag="R2")
            a9 = _act_raw(nc, mybir, AF.Rsqrt, R2[:], PG[:])
            p4 = nc.gpsimd.tensor_mul(RRST[:, 256:384], PTS[:], R2[:])

            # ---- fold weights+scales, then ONE f32r reduce matmul ----
            tvb = cp.tile([1, 1], f32)
            d10 = nc.vector.tensor_copy(out=tvb[:], in_=kb[0:1, SCL0:SCL0 + 1])
            RRS = wp.tile([128, 3 * JC], f32r, tag="RRS")
            d11 = nc.vector.tensor_mul(RRS[:], RRST[:],
                                       kb[:, SCL0:SCL0 + 3 * JC])
            nc.tensor.matmul(Fp[:], selr[:], RRS[:],
                             start=True, stop=True, **MM)

            # ---- tail: Vc+Vd combine, shift, one out DMA ----
            FS = wp.tile([G, 4 * JC], f32, tag="FS")
            a10 = nc.scalar.copy(out=FS[:, 0:3 * JC], in_=Fp[:])
            d12 = nc.vector.tensor_add(FS[:, 3 * JC:4 * JC],
                                       FS[:, JC:2 * JC],
                                       FS[:, 2 * JC:3 * JC])
            d13 = nc.vector.tensor_add(FS[:, JC:2 * JC],
                                       FS[:, 3 * JC:4 * JC],
                                       kb[0:G, CT0:CT0 + JC])
            nc.sync.dma_start(out=out_d[:], in_=FS[:, 0:2 * JC])

            for chain in ([a1, a2, a3, a4, a5, a6, a7, a8, a9, a10],
                          [d1, d2, d3, d4, d5, d6, d7, d8, d9, d10,
                           d11, d12, d13],
                          [p1, p2, p3, p4]):
                for prev, nxt in zip(chain, chain[1:]):
                    _add_dep_helper(nxt.ins, prev.ins, sync=False,
                                    reason="pin static engine order")

    if prune:
        _prune_redundant_waits(nc)
    return nc


def _get_nc():
    if "nc" not in _COMPILED:
        _COMPILED["nc"] = _build_nc()
    return _COMPILED["nc"]


def kernel(a, b, logcoef, shift, zs, _trace=False):
    from concourse.bass_utils import run_bass_kernel_spmd

    a = np.asarray(a)
    b = np.asarray(b)
    zs = np.asarray(zs)
    assert zs.shape == (B_TOTAL,)

    cstA_all, cstB_all, ckA_all, ckB_all = _build_host_tables(
        a, b, logcoef, shift, zs)

    in_maps = [
        {"cstA": cstA_all[c], "cstB": cstB_all[c],
         "ckA": ckA_all[c], "ckB": ckB_all[c]}
        for c in range(NCORES)
    ]

    nc = _get_nc()
    res = run_bass_kernel_spmd(nc, in_maps, core_ids=list(range(NCORES)),
                               trace=_trace)
    # out [G, 2*JC]: cols 0:128 = L, 128:256 = V, per group g
    outs = []
    for c in range(NCORES):
        o = res.results[c]["out"]
        outs.append(np.stack([o[:, 0:JC].reshape(BPC),
                              o[:, JC:2 * JC].reshape(BPC)]))
    out = np.concatenate(outs, axis=1)
    if _trace:
        kernel.last_exec_time_ns = res.exec_time_ns
        kernel.last_profile = res.profile_json
    return out.astype(np.float32)
